# revision 51
# baseline (speedup 1.0000x reference)
"""Trainium2 Bass kernel for a 16-head attention block with 2D axial RoPE.

Strategy: pure data-parallel over batch (32 batches -> 4 per NeuronCore),
bf16 compute, feature-major ("transposed") layouts throughout:
  - q/k produced feature-major by the QKV projection; v token-major
    directly (operand swap in the matmul).
  - RoPE: two elementwise muls (tables in SBUF) + a stream_shuffle
    partition pair-swap (DVE) + add. No PE permute matmul.
  - scoresT[m,n] per head (keys on partitions): row-packed K=64 matmul
    pairs; block-causal mask via multiplicative mask on slice 0 only
    (gpsimd); softmax without max subtraction (scores are O(1));
    sums via an appended ones-column on v.
  - normalization: fast approximate reciprocal (DVE custom op) + selector
    matmul broadcasting 1/sum across the 64 feature partitions per head,
    then elementwise mult.
  - proj computed feature-major (out = wp.T @ att), output DMA'd as
    [DIM, tokens]; the host transposes back.
  - Emission order is hand-scheduled for the in-order engine streams:
    previous-batch normalization is interleaved into the QKV phase, its
    projection matmuls fill the scores/AV dependency stalls, the last
    batch's normalization is split so half overlaps its own attention,
    and DMA issues are spread across sync/gpsimd with first-needed
    chunks front-loaded.
"""
import sys, os
sys.path.insert(0, "/opt/trn_rl_repo")
import numpy as np
import ml_dtypes

B, NTOK, DIM, H, HD = 32, 341, 1024, 16, 64
NCORES, BPC = 8, 4          # cores, batches per core
NP = 344                    # padded tokens per batch (bf16 pair aligned)
T = BPC * NP                # 1376 tokens per core
SCALES = [1, 2, 4, 8, 16]
PT_SEQ_LEN, THETA = 16, 10000.0
ROPE_DIM = HD // 2
MSL = [(0, 128), (128, 128), (256, 85)]   # m/token slices per batch
BF16 = ml_dtypes.bfloat16
SWAP_MASK = [i ^ 1 for i in range(32)]

_cache = {}


def _rope_tables():
    inv = 1.0 / (THETA ** (np.arange(0, ROPE_DIM, 2, dtype=np.float64) / ROPE_DIM))
    cos_list, sin_list = [], []
    for s in SCALES:
        t = np.arange(s, dtype=np.float64) / s * PT_SEQ_LEN
        f = np.outer(t, inv)
        f = np.repeat(f, 2, axis=-1)
        fy = np.broadcast_to(f[:, None, :], (s, s, ROPE_DIM))
        fx = np.broadcast_to(f[None, :, :], (s, s, ROPE_DIM))
        ff = np.concatenate([fy, fx], axis=-1).reshape(s * s, HD)
        cos_list.append(np.cos(ff))
        sin_list.append(np.sin(ff))
    cos = np.concatenate(cos_list, axis=0).astype(np.float32)  # [341, 64]
    sin = np.concatenate(sin_list, axis=0).astype(np.float32)
    return cos, sin


def _host_tables():
    cos, sin = _rope_tables()               # [341, 64]
    # sin2: sign pattern for rotate_half: q'[2i] = q[2i]c - q[2i+1]s ...
    sin2 = sin.copy()
    sin2[:, 0::2] = -sin[:, 0::2]
    # sinP[e] = sin2[e^1] (so that shuffle(q*sinP)[d] = q[d^1]*sin2[d])
    sinP = np.empty_like(sin2)
    sinP[:, 0::2] = sin2[:, 1::2]
    sinP[:, 1::2] = sin2[:, 0::2]
    cosT = np.zeros((HD, NP), np.float32)
    sinPT = np.zeros((HD, NP), np.float32)
    cosT[:, :NTOK] = cos.T
    sinPT[:, :NTOK] = sinP.T
    cos128 = np.vstack([cosT, cosT])        # [128, NP] two heads per tile
    sinP128 = np.vstack([sinPT, sinPT])
    scale = 1.0 / np.sqrt(HD)
    # tabs: cosq, sinq (scaled), cosk, sink
    tabs = np.concatenate(
        [cos128 * scale, sinP128 * scale, cos128, sinP128], axis=1
    )  # [128, 4*NP]
    ones = np.ones((1, NP), np.float32)
    return tabs.astype(BF16), ones.astype(BF16)


def _build(mask_mode, use_qkv_bias):
    import concourse.bass as bass
    import concourse.bacc as bacc
    import concourse.tile as tile
    from concourse import mybir

    f32, bf16 = mybir.dt.float32, mybir.dt.bfloat16
    nc = bacc.Bacc("TRN2", target_bir_lowering=False, debug=False)

    xt_d = nc.dram_tensor("xt", [DIM, T], bf16, kind="ExternalInput")
    wqk_d = nc.dram_tensor("wqk", [DIM, 2048], bf16, kind="ExternalInput")
    wv_d = nc.dram_tensor("wv", [DIM, 1024], bf16, kind="ExternalInput")
    wp_d = nc.dram_tensor("wp", [DIM, 1024], bf16, kind="ExternalInput")
    tabs_d = nc.dram_tensor("tabs", [128, 4 * NP], bf16, kind="ExternalInput")
    ones_d = nc.dram_tensor("ones", [1, NP], bf16, kind="ExternalInput")
    sel_d = nc.dram_tensor("sel", [16, 1024], bf16, kind="ExternalInput")
    sel8_d = nc.dram_tensor("sel8", [8, 1024], bf16, kind="ExternalInput")
    sel4_d = nc.dram_tensor("sel4", [4, 1024], bf16, kind="ExternalInput")
    use_mask = mask_mode in ("bc", "general")
    if mask_mode == "bc":
        # rank-4 additive block-causal mask: -1e9*[seg(m)>seg(n)] =
        # sum_j u4[j,m]*w4[j,n] (staircase), folded into the scores PSUM
        u4_d = nc.dram_tensor("u4", [4, 128], bf16, kind="ExternalInput")
        w4_d = nc.dram_tensor("w4", [4, 2 * 85], bf16, kind="ExternalInput")
    elif mask_mode == "general":
        mident_d = nc.dram_tensor("mident", [128, 128], bf16, kind="ExternalInput")
        maskg_d = nc.dram_tensor("maskg", [128, 2 * 3 * NP], bf16, kind="ExternalInput")
    if use_qkv_bias:
        qb_d = nc.dram_tensor("qb", [128, 16 * NP], bf16, kind="ExternalInput")  # rope'd q,k bias per f_tile
        vb_d = nc.dram_tensor("vb", [1, 1024], bf16, kind="ExternalInput")
    out_d = nc.dram_tensor("out", [DIM, BPC * NTOK], bf16, kind="ExternalOutput")

    with tile.TileContext(nc) as tc, \
         nc.allow_low_precision(reason="bf16 softmax stats; rel gate 2e-2"):
        with tc.tile_pool(name="res", bufs=1) as res, \
             tc.tile_pool(name="vp", bufs=3) as vpool, \
             tc.tile_pool(name="qkp", bufs=2) as qkpool, \
             tc.tile_pool(name="ro", bufs=3) as ropool, \
             tc.tile_pool(name="ex", bufs=2) as expool, \
             tc.tile_pool(name="avs", bufs=2) as avsp, \
             tc.tile_pool(name="st", bufs=3) as stpool, \
             tc.tile_pool(name="at", bufs=1) as atpool, \
             tc.tile_pool(name="ys", bufs=2) as yspool, \
             tc.tile_pool(name="uni", bufs=2, space="PSUM") as uni, \
             tc.tile_pool(name="av", bufs=2, space="PSUM") as avp, \
             tc.tile_pool(name="pf", bufs=2, space="PSUM") as pfp:

            # ---- resident loads ----
            # Issue serialization on one engine costs ~0.6us per dma_start,
            # so spread issues across sync/scalar/vector/gpsimd and put the
            # first-needed pieces (rope tables, f0/f1 weights, batch-0 x)
            # up front. gpsimd is idle until the first mask multiply, so it
            # takes the long non-urgent loads.
            xt = res.tile([128, 8, T], bf16)
            wqk = res.tile([128, 8, 2048], bf16)
            wv = res.tile([128, 8, 1024], bf16)
            wp = res.tile([128, 8, 1024], bf16)
            tabs = res.tile([128, 4, NP], bf16)
            sel = res.tile([16, 1024], bf16)
            sel8 = res.tile([8, 1024], bf16)
            sel4 = res.tile([4, 1024], bf16)
            def big(eng, dst, dram, cols):
                # one DMA covering all 8 c-chunks: dst [128, 8, ncols]
                src = dram.rearrange("(c p) t -> p c t", c=8)
                eng.dma_start(dst[:, 0:8, cols[0]:cols[1]],
                              src[:, :, cols[0]:cols[1]])

            # Bulk loads on ONE queue (gpsimd; DMA bandwidth is shared
            # across queues so parallelism doesn't help), one large multi-dim
            # DMA per piece, ordered by first need.
            big(nc.gpsimd, xt, xt_d, (0, NP))
            big(nc.gpsimd, wqk, wqk_d, (0, 256))
            big(nc.gpsimd, wqk, wqk_d, (256, 1024))
            nc.sync.dma_start(tabs[:], tabs_d[:])
            nc.sync.dma_start(sel[:], sel_d[:])
            nc.sync.dma_start(sel8[:], sel8_d[:])
            nc.sync.dma_start(sel4[:], sel4_d[:])
            if mask_mode == "bc":
                u4 = res.tile([4, 128], bf16)
                w4 = res.tile([4, 2, 85], bf16)
                nc.sync.dma_start(u4[:], u4_d[:])
                nc.sync.dma_start(w4[:], w4_d[:])
            elif mask_mode == "general":
                mident = res.tile([128, 128], bf16)
                maskg = res.tile([128, 2, 3, NP], bf16)
                nc.sync.dma_start(mident[:], mident_d[:])
                nc.sync.dma_start(maskg[:], maskg_d[:])
            if use_qkv_bias:
                ones = res.tile([1, NP], bf16)
                nc.sync.dma_start(ones[:], ones_d[:])
                vb = res.tile([1, 1024], bf16)
                nc.sync.dma_start(vb[:], vb_d[:])
            big(nc.gpsimd, wv, wv_d, (0, 512))
            big(nc.gpsimd, wv, wv_d, (512, 1024))
            big(nc.gpsimd, wqk, wqk_d, (1024, 2048))
            big(nc.gpsimd, xt, xt_d, (NP, T))
            big(nc.gpsimd, wp, wp_d, (0, 1024))

            pending = []

            def recip_fast(dst_rows, staged_t, rows):
                # staged sums are bf16 >= 1; cast to f32, approx 1/x
                # (~18 bits, 5x faster than reciprocal), cast back
                stf = stpool.tile([16, NP], f32, tag="stf", bufs=1,
                                  name="stf")
                nc.vector.tensor_copy(stf[0:rows, :], staged_t[:])
                rcf = stpool.tile([16, NP], f32, tag="rcf", bufs=1,
                                  name="rcf")
                nc.vector.reciprocal_approx_fast(rcf[0:rows, :],
                                                 stf[0:rows, :])
                nc.vector.tensor_copy(dst_rows, rcf[0:rows, :])

            def emit_recip(item, part=None):
                # part: None = all 16 heads; "a" = heads 0-7 (also allocates
                # att); "m"/"t" = head quarters 8-11 / 12-15 (tail interleave)
                if part is None or part == "a":
                    item["att"] = atpool.tile([128, 8, NP], bf16, name="att")
                if part is None:
                    item["rec"] = stpool.tile([16, NP], bf16, tag="rec",
                                              name="rec")
                    recip_fast(item["rec"][:], item["staged"], 16)
                else:
                    rows = 8 if part == "a" else 4
                    item["rec_" + part] = stpool.tile(
                        [rows, NP], bf16, tag="rec" + part, bufs=1,
                        name="rec_" + part)
                    recip_fast(item["rec_" + part][:],
                               item["staged_" + part], rows)

            def emit_norm_pair(item, p):
                # broadcast 1/sum for heads 2p,2p+1 into SBUF via a
                # replicating DMA (src row 0-stride), then normalize the AV
                # outputs into att (TTs split gpsimd/vector)
                if "rec" in item:
                    rech, base = item["rec"], 2 * p
                elif p < 4:
                    rech, base = item["rec_a"], 2 * p
                elif p < 6:
                    rech, base = item["rec_m"], 2 * (p - 4)
                else:
                    rech, base = item["rec_t"], 2 * (p - 6)
                prbs = ropool.tile([64, 2, NP], bf16, tag="prb", bufs=2,
                                   name="prbs")
                for hh in range(2):
                    src = rech[base + hh:base + hh + 1, :].unsqueeze(
                        1).broadcast_to((1, 64, NP))
                    nc.gpsimd.dma_start(prbs[:, hh, :], src)
                for hh in range(2):
                    h = 2 * p + hh
                    eng = nc.gpsimd if hh == 0 else nc.vector
                    eng.tensor_tensor(
                        item["att"][hh * 64:(hh + 1) * 64, p, :],
                        item["asb"][0:64, h, :],
                        prbs[0:64, hh, :],
                        mybir.AluOpType.mult)

            def make_proj_steps(item, fo, tail=False):
                # 9 lazily-emitted steps: 8 accumulating matmuls + ysb/DMA.
                # One DMA per fo group, alternating sync/gpsimd so neither
                # queue builds a backlog that outlives the last matmul.
                py = pfp.tile([128, 512], f32, tag="pf", name="py")

                def step(c):
                    nc.tensor.matmul(
                        py[:, 0:NP],
                        lhsT=wp[:, c, fo * 128:(fo + 1) * 128],
                        rhs=item["att"][:, c, :],
                        start=(c == 0), stop=(c == 7))
                    if c == 7:
                        ysb = yspool.tile([128, NP], bf16, name="ysb")
                        nc.scalar.copy(ysb[:, 0:NTOK], py[:, 0:NTOK])
                        ob = item["b"] * NTOK
                        eng = nc.sync if fo % 2 == 0 else nc.gpsimd
                        eng.dma_start(
                            out_d[fo * 128:(fo + 1) * 128, ob:ob + NTOK],
                            ysb[:, 0:NTOK])
                return step

            def emit_v_group(vt, boff, g):
                s, half = divmod(g, 2)
                t0, tsz = MSL[s]
                if half == 0:
                    vt[s] = vpool.tile([128, 16, 65], bf16, name=f"v{s}")
                v_s = vt[s]
                pv = pfp.tile([128, 512], f32, tag="pf", name="pv")
                for c in range(8):
                    nc.tensor.matmul(
                        pv[0:tsz, :],
                        lhsT=xt[:, c, boff + t0: boff + t0 + tsz],
                        rhs=wv[:, c, half * 512:(half + 1) * 512],
                        start=(c == 0), stop=(c == 7 and not use_qkv_bias))
                if use_qkv_bias:
                    nc.tensor.matmul(
                        pv[0:tsz, :],
                        lhsT=ones[0:1, 0:tsz],
                        rhs=vb[:, half * 512:(half + 1) * 512],
                        start=False, stop=True)
                nc.vector.tensor_copy(
                    v_s[0:tsz, half * 8:(half + 1) * 8, 0:64], pv[0:tsz, :])
                if half == 1:
                    nc.vector.memset(v_s[:, :, 64:65], 1.0)

            for b in range(BPC):
                boff = b * NP
                prev = pending.pop(0) if pending else None

                # ---- QKV + rope, interleaved with V groups and the previous
                # batch's normalization (PE never waits on a single chain) ----
                qk = qkpool.tile([128, 16, NP], bf16)
                vt = [None, None, None]
                for f in range(16):
                    if use_qkv_bias:
                        qbt = ropool.tile([128, NP], bf16, tag="qb", bufs=3,
                                          name="qbt")
                        nc.gpsimd.dma_start(
                            qbt[:], qb_d[:, f * NP:(f + 1) * NP])
                    pqkt = uni.tile([128, 2, 512], f32, tag="u", name="pqkt")
                    pqk = pqkt[:, 0, 0:NP]
                    for c in range(8):
                        nc.tensor.matmul(
                            pqk,
                            lhsT=wqk[:, c, f * 128:(f + 1) * 128],
                            rhs=xt[:, c, boff: boff + NP],
                            start=(c == 0), stop=(c == 7))
                    is_q = f < 8
                    cosT = tabs[:, 0, :] if is_q else tabs[:, 2, :]
                    sinT = tabs[:, 1, :] if is_q else tabs[:, 3, :]
                    qsb = ropool.tile([128, NP], bf16, tag="qs")
                    nc.scalar.copy(qsb[:], pqk)
                    tmul = ropool.tile([128, NP], bf16, tag="tm")
                    umul = ropool.tile([128, NP], bf16, tag="um")
                    nc.vector.tensor_tensor(tmul[:], qsb[:], cosT, mybir.AluOpType.mult)
                    nc.vector.tensor_tensor(umul[:], qsb[:], sinT, mybir.AluOpType.mult)
                    usw = ropool.tile([128, NP], bf16, tag="us")
                    nc.vector.stream_shuffle(usw[:], umul[:], SWAP_MASK)
                    if use_qkv_bias:
                        tqb = ropool.tile([128, NP], bf16, tag="tb")
                        nc.vector.tensor_tensor(tqb[:], tmul[:], usw[:],
                                                mybir.AluOpType.add)
                        nc.vector.tensor_tensor(qk[:, f, :], tqb[:], qbt[:],
                                                mybir.AluOpType.add)
                    else:
                        nc.vector.tensor_tensor(qk[:, f, :], tmul[:], usw[:],
                                                mybir.AluOpType.add)
                    if f in (3, 5, 7, 9, 11, 13):
                        emit_v_group(vt, boff, (f - 3) // 2)
                    if prev is not None:
                        if f == 3:
                            emit_recip(prev)
                        elif 8 <= f < 12:
                            emit_norm_pair(prev, 2 * (f - 8))
                            emit_norm_pair(prev, 2 * (f - 8) + 1)

                # ---- attention: scores (hh-merged PSUM tiles, mask folded in
                # via rank-4/identity matmuls), one exp per (p, slice), AV;
                # previous batch's proj matmuls spread through as PE filler ----
                asb = avsp.tile([65, 16, NP], bf16, name="asb")
                if b == BPC - 1:
                    staged_a = stpool.tile([8, NP], bf16, tag="sga", bufs=1)
                    staged_m = stpool.tile([4, NP], bf16, tag="sgm", bufs=1)
                    staged_t = stpool.tile([4, NP], bf16, tag="sgt", bufs=1)
                    self_item = {"b": b, "staged_a": staged_a,
                                 "staged_m": staged_m, "staged_t": staged_t,
                                 "asb": asb}
                else:
                    staged = stpool.tile([16, NP], bf16, tag="staged")
                    self_item = {"b": b, "staged": staged, "asb": asb}
                for p in range(8):
                    pj = make_proj_steps(prev, p) if prev is not None else None
                    ex = expool.tile([128, 2, 3, NP], bf16, tag="ex", name="ex")
                    for si, (m0, msz) in enumerate(MSL):
                        # block-causal: slices 1,2 (keys >= 128, all in the last
                        # segment) only attend queries n >= 85; no mask needed.
                        n0, nsz = (85, NP - 85) if (mask_mode == "bc" and si > 0) else (0, NP)
                        slice_mask = (mask_mode == "general") or (mask_mode == "bc" and si == 0)
                        ps = uni.tile([128, 2, 512], f32, tag="u", name="ps")
                        for hh in range(2):
                            r0 = hh * 64
                            nc.tensor.matmul(
                                ps[0:msz, hh, n0:n0 + nsz],
                                lhsT=qk[r0:r0 + 64, 8 + p, m0:m0 + msz],
                                rhs=qk[r0:r0 + 64, p, n0:n0 + nsz],
                                start=True, stop=not slice_mask,
                                tile_position=(r0, 0))
                        if slice_mask:
                            for hh in range(2):
                                if mask_mode == "bc":
                                    nc.tensor.matmul(
                                        ps[0:128, hh, 0:85],
                                        lhsT=u4[0:4, 0:128],
                                        rhs=w4[0:4, hh, 0:85],
                                        start=False, stop=True,
                                        tile_position=(0, 0))
                                else:
                                    nc.tensor.matmul(
                                        ps[0:msz, hh, n0:n0 + nsz],
                                        lhsT=mident[0:msz, 0:msz],
                                        rhs=maskg[0:msz, hh, si, n0:n0 + nsz],
                                        start=False, stop=True,
                                        tile_position=(0, 0))
                        nc.scalar.activation(
                            ex[0:msz, 0:2, si, n0:n0 + nsz],
                            ps[0:msz, 0:2, n0:n0 + nsz],
                            mybir.ActivationFunctionType.Exp)
                        if pj is not None:
                            pj(2 * si)
                            pj(2 * si + 1)
                    for hh in range(2):
                        h = 2 * p + hh
                        pav = avp.tile([128, 512], f32, tag="av", name="pav")
                        for si, (m0, msz) in enumerate(MSL):
                            n0, nsz = (85, NP - 85) if (mask_mode == "bc" and si > 0) else (0, NP)
                            nc.tensor.matmul(
                                pav[0:65, n0:n0 + nsz],
                                lhsT=vt[si][0:msz, h, :],
                                rhs=ex[0:msz, hh, si, n0:n0 + nsz],
                                start=(si == 0), stop=(si == 2))
                        if pj is not None:
                            pj(6 + hh)
                        nc.vector.tensor_copy(asb[:, h, :], pav[0:65, 0:NP])
                    # last batch: fold most of its own normalization into the
                    # attention phase (shrinks the exposed tail): heads 0-7
                    # staged at p3, quarters 8-11 at p5; pairs 0-5 normalized
                    # by the end of p7.
                    if b == BPC - 1:
                        if p == 3:
                            nc.sync.dma_start(staged_a[:],
                                              asb[64:65, 0:8, 0:NP])
                        elif p == 5:
                            emit_recip(self_item, part="a")
                            nc.sync.dma_start(staged_m[:],
                                              asb[64:65, 8:12, 0:NP])
                        elif p == 6:
                            emit_norm_pair(self_item, 0)
                            emit_norm_pair(self_item, 1)
                            emit_recip(self_item, part="m")
                        elif p == 7:
                            emit_norm_pair(self_item, 2)
                            emit_norm_pair(self_item, 3)
                            emit_norm_pair(self_item, 4)
                            emit_norm_pair(self_item, 5)
                if b == BPC - 1:
                    nc.sync.dma_start(staged_t[:], asb[64:65, 12:16, 0:NP])
                else:
                    nc.sync.dma_start(staged[:], asb[64:65, 0:16, 0:NP])
                pending.append(self_item)

            # ---- tail: the last batch's remaining normalization + proj ----
            while pending:
                item = pending.pop(0)
                if "rec_a" in item:
                    # pairs 0-5 were normalized during the attention phase;
                    # fo0/fo1's first six c-steps run while the last quarter's
                    # reciprocal + sel matmuls complete
                    emit_recip(item, part="t")
                    pj0 = make_proj_steps(item, 0, tail=True)
                    pj1 = make_proj_steps(item, 1, tail=True)
                    for c in range(6):
                        pj0(c)
                    emit_norm_pair(item, 6)
                    for c in range(6):
                        pj1(c)
                    emit_norm_pair(item, 7)
                    for c in range(6, 8):
                        pj0(c)
                    for c in range(6, 8):
                        pj1(c)
                    for p in range(2, 8):
                        pj = make_proj_steps(item, p, tail=True)
                        for c in range(8):
                            pj(c)
                else:
                    emit_recip(item)
                    for p in range(8):
                        emit_norm_pair(item, p)
                    for p in range(8):
                        pj = make_proj_steps(item, p, tail=True)
                        for c in range(8):
                            pj(c)
    nc.finalize()
    return nc


def _get_nc(mask_mode, use_qkv_bias):
    key = (mask_mode, use_qkv_bias)
    if key not in _cache:
        _cache[key] = _build(mask_mode, use_qkv_bias)
    return _cache[key]


def _bc_mask():
    seg = np.concatenate([np.full(s * s, i, dtype=np.int64) for i, s in enumerate(SCALES)])
    allow = seg[:, None] >= seg[None, :]
    return np.where(allow, 0.0, -1e9).astype(np.float32)[None, None]


def _prep_core_inputs(x, mask, qkv_w, qkv_b, proj_w, proj_b):
    tabs, ones = _host_tables()
    mf = mask.astype(np.float32)
    if not np.any(mf != 0):
        mask_mode = "none"
    elif np.array_equal(mf, _bc_mask()):
        mask_mode = "bc"
    else:
        mask_mode = "general"
    use_mask = mask_mode != "none"
    use_qb = bool(np.any(qkv_b != 0))

    wqkT = qkv_w.astype(np.float32).T.astype(BF16)      # [1024, 3072]
    wqk = np.ascontiguousarray(wqkT[:, :2048])
    wv = np.ascontiguousarray(wqkT[:, 2048:])
    wpT = np.ascontiguousarray(proj_w.astype(np.float32).T.astype(BF16))

    sel = np.zeros((16, 1024), np.float32)
    for h in range(16):
        sel[h, h * 64:(h + 1) * 64] = 1.0
    sel8 = np.zeros((8, 1024), np.float32)
    sel4 = np.zeros((4, 1024), np.float32)
    for cg in range(1024):
        p, r = cg // 128, cg % 128
        sel8[2 * (p % 4) + r // 64, cg] = 1.0
        sel4[2 * (p % 2) + r // 64, cg] = 1.0
    common = {"wqk": wqk, "wv": wv, "wp": wpT, "tabs": np.ascontiguousarray(tabs),
              "ones": np.ascontiguousarray(ones), "sel": sel.astype(BF16),
              "sel8": sel8.astype(BF16), "sel4": sel4.astype(BF16)}
    seg = np.concatenate([np.full(s * s, i, dtype=np.int64) for i, s in enumerate(SCALES)])
    if mask_mode == "bc":
        # staircase rank-4 additive mask (keys m<128 x queries n<85 only;
        # elsewhere block-causality holds by slice construction)
        u4 = np.zeros((4, 128), np.float32)
        w4 = np.zeros((4, 2, 85), np.float32)
        for j in range(4):
            u4[j, :] = np.where(seg[:128] >= j + 1, -1e9, 0.0)
            w4[j, :, :] = np.where(seg[:85] == j, 1.0, 0.0)[None, :]
        common["u4"] = u4.astype(BF16)
        common["w4"] = w4.reshape(4, 2 * 85).astype(BF16)
    elif mask_mode == "general":
        mT = mask[0, 0].astype(np.float32).T            # [keys, queries]
        mg = np.zeros((128, 2, 3, NP), np.float32)
        for s in range(3):
            ks = min(128, NTOK - s * 128)
            mg[:ks, :, s, :NTOK] = mT[s * 128:s * 128 + ks, None, :]
        common["mident"] = np.eye(128, dtype=np.float32).astype(BF16)
        common["maskg"] = mg.reshape(128, 2 * 3 * NP).astype(BF16)
    if use_qb:
        cos, sin = _rope_tables()
        sin2 = sin.copy(); sin2[:, 0::2] = -sin[:, 0::2]
        scale = 1.0 / np.sqrt(HD)
        qb_full = np.zeros((128, 16 * NP), np.float32)
        bq = qkv_b[:2048].astype(np.float32)
        for f in range(16):
            is_q = f < 8
            sc = scale if is_q else 1.0
            for hh in range(2):
                hvec = bq[f * 128 + hh * 64: f * 128 + (hh + 1) * 64]  # [64]
                hswap = hvec.reshape(-1, 2)[:, ::-1].reshape(-1)
                rb = cos * hvec[None, :] + sin2 * hswap[None, :]       # [341,64]
                qb_full[hh * 64:(hh + 1) * 64, f * NP: f * NP + NTOK] = sc * rb.T
        common["qb"] = qb_full.astype(BF16)
        common["vb"] = qkv_b[2048:].astype(np.float32).astype(BF16)[None, :]

    in_maps = []
    xf = x.astype(np.float32)
    for core in range(NCORES):
        xc = xf[core * BPC:(core + 1) * BPC]            # [4, 341, 1024]
        xp = np.zeros((BPC, NP, DIM), np.float32)
        xp[:, :NTOK, :] = xc
        xT = xp.reshape(BPC * NP, DIM).T                # [1024, 1376]
        m = dict(common)
        m["xt"] = np.ascontiguousarray(xT.astype(BF16))
        in_maps.append(m)
    return in_maps, mask_mode, use_qb


def kernel(x, mask, qkv_w, qkv_b, proj_w, proj_b, _trace=False):
    from concourse.bass_utils import run_bass_kernel_spmd
    x, mask, qkv_w, qkv_b, proj_w, proj_b = (
        np.asarray(t) for t in (x, mask, qkv_w, qkv_b, proj_w, proj_b))
    in_maps, mask_mode, use_qb = _prep_core_inputs(
        x, mask, qkv_w, qkv_b, proj_w, proj_b)
    nc = _get_nc(mask_mode, use_qb)
    res = run_bass_kernel_spmd(nc, in_maps, core_ids=list(range(NCORES)),
                               trace=_trace)
    out = np.empty((B, NTOK, DIM), np.float32)
    for core in range(NCORES):
        y = res.results[core]["out"].astype(np.float32)  # [DIM, BPC*NTOK]
        out[core * BPC:(core + 1) * BPC] = (
            y.T.reshape(BPC, NTOK, DIM))
    pb = proj_b.astype(np.float32)
    if np.any(pb != 0):
        out += pb[None, None, :]
    kernel._last_exec_time_ns = res.exec_time_ns
    if res.instructions_and_trace is not None:
        kernel._last_trace_path = res.instructions_and_trace[1]
    return out



# revision 54
# speedup vs baseline: 1.2450x; 1.2450x over previous
"""Trainium2 Bass kernel for a 16-head attention block with 2D axial RoPE.

Strategy: pure data-parallel over batch (32 batches -> 4 per NeuronCore),
bf16 compute, feature-major ("transposed") layouts throughout:
  - q/k produced feature-major by the QKV projection; v token-major
    directly (operand swap in the matmul).
  - RoPE: two elementwise muls (tables in SBUF) + a stream_shuffle
    partition pair-swap (DVE) + add. No PE permute matmul.
  - scoresT[m,n] per head (keys on partitions): row-packed K=64 matmul
    pairs; block-causal mask via multiplicative mask on slice 0 only
    (gpsimd); softmax without max subtraction (scores are O(1));
    sums via an appended ones-column on v.
  - normalization: fast approximate reciprocal (DVE custom op) + selector
    matmul broadcasting 1/sum across the 64 feature partitions per head,
    then elementwise mult.
  - proj computed feature-major (out = wp.T @ att), output DMA'd as
    [DIM, tokens]; the host transposes back.
  - Emission order is hand-scheduled for the in-order engine streams:
    previous-batch normalization is interleaved into the QKV phase, its
    projection matmuls fill the scores/AV dependency stalls, the last
    batch's normalization is split so half overlaps its own attention,
    and DMA issues are spread across sync/gpsimd with first-needed
    chunks front-loaded.
"""
import sys, os
sys.path.insert(0, "/opt/trn_rl_repo")
import numpy as np
import ml_dtypes

B, NTOK, DIM, H, HD = 32, 341, 1024, 16, 64
NCORES, BPC = 8, 4          # cores, batches per core
NP = 344                    # padded tokens per batch (bf16 pair aligned)
T = BPC * NP                # 1376 tokens per core
SCALES = [1, 2, 4, 8, 16]
PT_SEQ_LEN, THETA = 16, 10000.0
ROPE_DIM = HD // 2
MSL = [(0, 128), (128, 128), (256, 85)]   # m/token slices per batch
BF16 = ml_dtypes.bfloat16
SWAP_MASK = [i ^ 1 for i in range(32)]

_cache = {}


def _rope_tables():
    inv = 1.0 / (THETA ** (np.arange(0, ROPE_DIM, 2, dtype=np.float64) / ROPE_DIM))
    cos_list, sin_list = [], []
    for s in SCALES:
        t = np.arange(s, dtype=np.float64) / s * PT_SEQ_LEN
        f = np.outer(t, inv)
        f = np.repeat(f, 2, axis=-1)
        fy = np.broadcast_to(f[:, None, :], (s, s, ROPE_DIM))
        fx = np.broadcast_to(f[None, :, :], (s, s, ROPE_DIM))
        ff = np.concatenate([fy, fx], axis=-1).reshape(s * s, HD)
        cos_list.append(np.cos(ff))
        sin_list.append(np.sin(ff))
    cos = np.concatenate(cos_list, axis=0).astype(np.float32)  # [341, 64]
    sin = np.concatenate(sin_list, axis=0).astype(np.float32)
    return cos, sin


def _host_tables():
    cos, sin = _rope_tables()               # [341, 64]
    # sin2: sign pattern for rotate_half: q'[2i] = q[2i]c - q[2i+1]s ...
    sin2 = sin.copy()
    sin2[:, 0::2] = -sin[:, 0::2]
    # sinP[e] = sin2[e^1] (so that shuffle(q*sinP)[d] = q[d^1]*sin2[d])
    sinP = np.empty_like(sin2)
    sinP[:, 0::2] = sin2[:, 1::2]
    sinP[:, 1::2] = sin2[:, 0::2]
    cosT = np.zeros((HD, NP), np.float32)
    sinPT = np.zeros((HD, NP), np.float32)
    cosT[:, :NTOK] = cos.T
    sinPT[:, :NTOK] = sinP.T
    cos128 = np.vstack([cosT, cosT])        # [128, NP] two heads per tile
    sinP128 = np.vstack([sinPT, sinPT])
    scale = 1.0 / np.sqrt(HD)
    # tabs: cosq, sinq (scaled), cosk, sink
    tabs = np.concatenate(
        [cos128 * scale, sinP128 * scale, cos128, sinP128], axis=1
    )  # [128, 4*NP]
    # duplicate each table for the f-pair rope ([128, 4, 2, NP] layout)
    tabs = np.repeat(tabs.reshape(128, 4, 1, NP), 2, axis=2).reshape(128, -1)
    ones = np.ones((1, NP), np.float32)
    return tabs.astype(BF16), ones.astype(BF16)


def _build(mask_mode, use_qkv_bias):
    import concourse.bass as bass
    import concourse.bacc as bacc
    import concourse.tile as tile
    from concourse import mybir

    f32, bf16 = mybir.dt.float32, mybir.dt.bfloat16
    nc = bacc.Bacc("TRN2", target_bir_lowering=False, debug=False)

    xt_d = nc.dram_tensor("xt", [DIM, T], bf16, kind="ExternalInput")
    wqk_d = nc.dram_tensor("wqk", [DIM, 2048], bf16, kind="ExternalInput")
    wv_d = nc.dram_tensor("wv", [DIM, 1024], bf16, kind="ExternalInput")
    wp_d = nc.dram_tensor("wp", [DIM, 1024], bf16, kind="ExternalInput")
    tabs_d = nc.dram_tensor("tabs", [128, 4 * 2 * NP], bf16, kind="ExternalInput")
    ones_d = nc.dram_tensor("ones", [1, NP], bf16, kind="ExternalInput")
    sel_d = nc.dram_tensor("sel", [16, 1024], bf16, kind="ExternalInput")
    sel8_d = nc.dram_tensor("sel8", [8, 1024], bf16, kind="ExternalInput")
    sel4_d = nc.dram_tensor("sel4", [4, 1024], bf16, kind="ExternalInput")
    use_mask = mask_mode in ("bc", "general")
    if mask_mode == "bc":
        # rank-4 additive block-causal mask: -1e9*[seg(m)>seg(n)] =
        # sum_j u4[j,m]*w4[j,n] (staircase), folded into the scores PSUM
        u4_d = nc.dram_tensor("u4", [4, 128], bf16, kind="ExternalInput")
        w4_d = nc.dram_tensor("w4", [4, 2 * 85], bf16, kind="ExternalInput")
    elif mask_mode == "general":
        mident_d = nc.dram_tensor("mident", [128, 128], bf16, kind="ExternalInput")
        maskg_d = nc.dram_tensor("maskg", [128, 2 * 3 * NP], bf16, kind="ExternalInput")
    if use_qkv_bias:
        qb_d = nc.dram_tensor("qb", [128, 16 * NP], bf16, kind="ExternalInput")  # rope'd q,k bias per f_tile
        vb_d = nc.dram_tensor("vb", [1, 1024], bf16, kind="ExternalInput")
    out_d = nc.dram_tensor("out", [DIM, BPC * NTOK], bf16, kind="ExternalOutput")

    with tile.TileContext(nc) as tc, \
         nc.allow_low_precision(reason="bf16 softmax stats; rel gate 2e-2"):
        with tc.tile_pool(name="res", bufs=1) as res, \
             tc.tile_pool(name="vp", bufs=3) as vpool, \
             tc.tile_pool(name="qkp", bufs=2) as qkpool, \
             tc.tile_pool(name="ro", bufs=3) as ropool, \
             tc.tile_pool(name="ex", bufs=2) as expool, \
             tc.tile_pool(name="avs", bufs=2) as avsp, \
             tc.tile_pool(name="st", bufs=3) as stpool, \
             tc.tile_pool(name="at", bufs=1) as atpool, \
             tc.tile_pool(name="ys", bufs=2) as yspool, \
             tc.tile_pool(name="uni", bufs=2, space="PSUM") as uni, \
             tc.tile_pool(name="av", bufs=2, space="PSUM") as avp, \
             tc.tile_pool(name="pf", bufs=2, space="PSUM") as pfp:

            # ---- resident loads ----
            # Issue serialization on one engine costs ~0.6us per dma_start,
            # so spread issues across sync/scalar/vector/gpsimd and put the
            # first-needed pieces (rope tables, f0/f1 weights, batch-0 x)
            # up front. gpsimd is idle until the first mask multiply, so it
            # takes the long non-urgent loads.
            xt = res.tile([128, 8, T], bf16)
            wqk = res.tile([128, 8, 2048], bf16)
            wv = res.tile([128, 8, 1024], bf16)
            wp = res.tile([128, 8, 1024], bf16)
            tabs2 = res.tile([128, 4, 2, NP], bf16)
            sel = res.tile([16, 1024], bf16)
            sel8 = res.tile([8, 1024], bf16)
            sel4 = res.tile([4, 1024], bf16)
            def big(eng, dst, dram, cols):
                # one DMA covering all 8 c-chunks: dst [128, 8, ncols]
                src = dram.rearrange("(c p) t -> p c t", c=8)
                eng.dma_start(dst[:, 0:8, cols[0]:cols[1]],
                              src[:, :, cols[0]:cols[1]])

            # Bulk loads on ONE queue (gpsimd; DMA bandwidth is shared
            # across queues so parallelism doesn't help), one large multi-dim
            # DMA per piece, ordered by first need.
            big(nc.gpsimd, xt, xt_d, (0, NP))
            big(nc.gpsimd, wqk, wqk_d, (0, 256))
            big(nc.gpsimd, wqk, wqk_d, (256, 1024))
            nc.sync.dma_start(tabs2[:], tabs_d[:])
            nc.sync.dma_start(sel[:], sel_d[:])
            nc.sync.dma_start(sel8[:], sel8_d[:])
            nc.sync.dma_start(sel4[:], sel4_d[:])
            if mask_mode == "bc":
                u4 = res.tile([4, 128], bf16)
                w4 = res.tile([4, 2, 85], bf16)
                nc.sync.dma_start(u4[:], u4_d[:])
                nc.sync.dma_start(w4[:], w4_d[:])
            elif mask_mode == "general":
                mident = res.tile([128, 128], bf16)
                maskg = res.tile([128, 2, 3, NP], bf16)
                nc.sync.dma_start(mident[:], mident_d[:])
                nc.sync.dma_start(maskg[:], maskg_d[:])
            if use_qkv_bias:
                ones = res.tile([1, NP], bf16)
                nc.sync.dma_start(ones[:], ones_d[:])
                vb = res.tile([1, 1024], bf16)
                nc.sync.dma_start(vb[:], vb_d[:])
            big(nc.gpsimd, wv, wv_d, (0, 512))
            big(nc.gpsimd, wv, wv_d, (512, 1024))
            big(nc.gpsimd, wqk, wqk_d, (1024, 2048))
            big(nc.gpsimd, xt, xt_d, (NP, T))
            big(nc.gpsimd, wp, wp_d, (0, 1024))

            pending = []

            def recip_fast(dst_rows, staged_t, rows):
                # staged sums are bf16 >= 1; cast to f32, approx 1/x
                # (~18 bits, 5x faster than reciprocal), cast back
                stf = stpool.tile([16, NP], f32, tag="stf", bufs=1,
                                  name="stf")
                nc.vector.tensor_copy(stf[0:rows, :], staged_t[:])
                rcf = stpool.tile([16, NP], f32, tag="rcf", bufs=1,
                                  name="rcf")
                nc.vector.reciprocal_approx_fast(rcf[0:rows, :],
                                                 stf[0:rows, :])
                nc.vector.tensor_copy(dst_rows, rcf[0:rows, :])

            def emit_recip(item, part=None):
                # part: None = all 16 heads; "a" = heads 0-7 (also allocates
                # att); "m"/"t" = head quarters 8-11 / 12-15 (tail interleave)
                if part is None or part == "a":
                    item["att"] = atpool.tile([128, 8, NP], bf16, name="att")
                if part is None:
                    item["rec"] = stpool.tile([16, NP], bf16, tag="rec",
                                              name="rec")
                    recip_fast(item["rec"][:], item["staged"], 16)
                else:
                    rows = 8 if part == "a" else 4
                    item["rec_" + part] = stpool.tile(
                        [rows, NP], bf16, tag="rec" + part, bufs=1,
                        name="rec_" + part)
                    recip_fast(item["rec_" + part][:],
                               item["staged_" + part], rows)

            def emit_norm_pair(item, p):
                # broadcast 1/sum for heads 2p,2p+1 (PE selector matmul) and
                # normalize the AV outputs into att (DVE)
                prb = avp.tile([128, 512], f32, tag="av", name="prb")
                if "rec" in item:
                    lhsT, rech = sel[0:16, 2 * p * 64:(2 * p + 2) * 64], item["rec"]
                elif p < 4:
                    lhsT, rech = sel8[0:8, 2 * p * 64:(2 * p + 2) * 64], item["rec_a"]
                else:
                    lhsT = sel4[0:4, 2 * p * 64:(2 * p + 2) * 64]
                    rech = item["rec_m"] if p < 6 else item["rec_t"]
                nc.tensor.matmul(prb[:, 0:NP], lhsT=lhsT, rhs=rech[:],
                                 start=True, stop=True)
                for hh in range(2):
                    h = 2 * p + hh
                    nc.vector.tensor_tensor(
                        item["att"][hh * 64:(hh + 1) * 64, p, :],
                        item["asb"][0:64, h, :], prb[hh * 64:(hh + 1) * 64, 0:NP],
                        mybir.AluOpType.mult)

            def make_proj_steps(item, fo, tail=False):
                # 9 lazily-emitted steps: 8 accumulating matmuls + ysb/DMA.
                # One DMA per fo group, alternating sync/gpsimd so neither
                # queue builds a backlog that outlives the last matmul.
                py = pfp.tile([128, 512], f32, tag="pf", name="py")

                def step(c):
                    nc.tensor.matmul(
                        py[:, 0:NP],
                        lhsT=wp[:, c, fo * 128:(fo + 1) * 128],
                        rhs=item["att"][:, c, :],
                        start=(c == 0), stop=(c == 7))
                    if c == 7:
                        ysb = yspool.tile([128, NP], bf16, name="ysb")
                        nc.scalar.copy(ysb[:, 0:NTOK], py[:, 0:NTOK])
                        ob = item["b"] * NTOK
                        eng = nc.sync if fo % 2 == 0 else nc.gpsimd
                        eng.dma_start(
                            out_d[fo * 128:(fo + 1) * 128, ob:ob + NTOK],
                            ysb[:, 0:NTOK])
                return step

            def emit_v_group(vt, boff, g):
                s, half = divmod(g, 2)
                t0, tsz = MSL[s]
                if half == 0:
                    vt[s] = vpool.tile([128, 16, 65], bf16, name=f"v{s}")
                v_s = vt[s]
                pv = pfp.tile([128, 512], f32, tag="pf", name="pv")
                for c in range(8):
                    nc.tensor.matmul(
                        pv[0:tsz, :],
                        lhsT=xt[:, c, boff + t0: boff + t0 + tsz],
                        rhs=wv[:, c, half * 512:(half + 1) * 512],
                        start=(c == 0), stop=(c == 7 and not use_qkv_bias))
                if use_qkv_bias:
                    nc.tensor.matmul(
                        pv[0:tsz, :],
                        lhsT=ones[0:1, 0:tsz],
                        rhs=vb[:, half * 512:(half + 1) * 512],
                        start=False, stop=True)
                nc.vector.tensor_copy(
                    v_s[0:tsz, half * 8:(half + 1) * 8, 0:64], pv[0:tsz, :])
                if half == 1:
                    nc.vector.memset(v_s[:, :, 64:65], 1.0)

            for b in range(BPC):
                boff = b * NP
                prev = pending.pop(0) if pending else None

                # ---- QKV + rope, interleaved with V groups and the previous
                # batch's normalization (PE never waits on a single chain) ----
                qk = qkpool.tile([128, 16, NP], bf16)
                vt = [None, None, None]
                # f-tiles processed in PAIRS: one rope op set per pair on
                # [128, 2, NP] APs (tables duplicated host-side) halves the
                # vector instruction count in this vector-saturated phase.
                for fp in range(8):
                    if use_qkv_bias:
                        qbt = ropool.tile([128, 2, NP], bf16, tag="qb", bufs=2,
                                          name="qbt")
                        nc.gpsimd.dma_start(
                            qbt[:], qb_d[:, 2 * fp * NP:(2 * fp + 2) * NP])
                    qsb = ropool.tile([128, 2, NP], bf16, tag="qs", bufs=2,
                                      name="qsb")
                    for j in range(2):
                        f = 2 * fp + j
                        pqkt = uni.tile([128, 2, 512], f32, tag="u", name="pqkt")
                        pqk = pqkt[:, 0, 0:NP]
                        for c in range(8):
                            nc.tensor.matmul(
                                pqk,
                                lhsT=wqk[:, c, f * 128:(f + 1) * 128],
                                rhs=xt[:, c, boff: boff + NP],
                                start=(c == 0), stop=(c == 7))
                        nc.scalar.copy(qsb[:, j, :], pqk)
                        # interleaves at f granularity
                        f = 2 * fp + j
                        if f in (3, 5, 7, 9, 11, 13):
                            emit_v_group(vt, boff, (f - 3) // 2)
                        if prev is not None:
                            if f == 3:
                                emit_recip(prev)
                            elif 8 <= f < 12:
                                emit_norm_pair(prev, 2 * (f - 8))
                                emit_norm_pair(prev, 2 * (f - 8) + 1)
                    is_q = fp < 4
                    cosT = tabs2[:, 0, :, :] if is_q else tabs2[:, 2, :, :]
                    sinT = tabs2[:, 1, :, :] if is_q else tabs2[:, 3, :, :]
                    tmul = ropool.tile([128, 2, NP], bf16, tag="tm", bufs=2,
                                       name="tmul")
                    umul = ropool.tile([128, 2, NP], bf16, tag="um", bufs=2,
                                       name="umul")
                    nc.vector.tensor_tensor(tmul[:], qsb[:], cosT, mybir.AluOpType.mult)
                    nc.vector.tensor_tensor(umul[:], qsb[:], sinT, mybir.AluOpType.mult)
                    usw = ropool.tile([128, 2, NP], bf16, tag="us", bufs=2,
                                      name="usw")
                    nc.vector.stream_shuffle(usw[:], umul[:], SWAP_MASK)
                    if use_qkv_bias:
                        tqb = ropool.tile([128, 2, NP], bf16, tag="tb", bufs=2,
                                          name="tqb")
                        nc.vector.tensor_tensor(tqb[:], tmul[:], usw[:],
                                                mybir.AluOpType.add)
                        nc.vector.tensor_tensor(qk[:, 2 * fp:2 * fp + 2, :],
                                                tqb[:], qbt[:],
                                                mybir.AluOpType.add)
                    else:
                        nc.vector.tensor_tensor(qk[:, 2 * fp:2 * fp + 2, :],
                                                tmul[:], usw[:],
                                                mybir.AluOpType.add)

                # ---- attention: scores (hh-merged PSUM tiles, mask folded in
                # via rank-4/identity matmuls), one exp per (p, slice), AV;
                # previous batch's proj matmuls spread through as PE filler ----
                asb = avsp.tile([65, 16, NP], bf16, name="asb")
                if b == BPC - 1:
                    staged_a = stpool.tile([8, NP], bf16, tag="sga", bufs=1)
                    staged_m = stpool.tile([4, NP], bf16, tag="sgm", bufs=1)
                    staged_t = stpool.tile([4, NP], bf16, tag="sgt", bufs=1)
                    self_item = {"b": b, "staged_a": staged_a,
                                 "staged_m": staged_m, "staged_t": staged_t,
                                 "asb": asb}
                else:
                    staged = stpool.tile([16, NP], bf16, tag="staged")
                    self_item = {"b": b, "staged": staged, "asb": asb}
                for p in range(8):
                    pj = make_proj_steps(prev, p) if prev is not None else None
                    ex = expool.tile([128, 2, 3, NP], bf16, tag="ex", name="ex")
                    for si, (m0, msz) in enumerate(MSL):
                        # block-causal: slices 1,2 (keys >= 128, all in the last
                        # segment) only attend queries n >= 85; no mask needed.
                        n0, nsz = (85, NP - 85) if (mask_mode == "bc" and si > 0) else (0, NP)
                        slice_mask = (mask_mode == "general") or (mask_mode == "bc" and si == 0)
                        ps = uni.tile([128, 2, 512], f32, tag="u", name="ps")
                        for hh in range(2):
                            r0 = hh * 64
                            nc.tensor.matmul(
                                ps[0:msz, hh, n0:n0 + nsz],
                                lhsT=qk[r0:r0 + 64, 8 + p, m0:m0 + msz],
                                rhs=qk[r0:r0 + 64, p, n0:n0 + nsz],
                                start=True, stop=not slice_mask,
                                tile_position=(r0, 0))
                        if slice_mask:
                            for hh in range(2):
                                if mask_mode == "bc":
                                    nc.tensor.matmul(
                                        ps[0:128, hh, 0:85],
                                        lhsT=u4[0:4, 0:128],
                                        rhs=w4[0:4, hh, 0:85],
                                        start=False, stop=True,
                                        tile_position=(0, 0))
                                else:
                                    nc.tensor.matmul(
                                        ps[0:msz, hh, n0:n0 + nsz],
                                        lhsT=mident[0:msz, 0:msz],
                                        rhs=maskg[0:msz, hh, si, n0:n0 + nsz],
                                        start=False, stop=True,
                                        tile_position=(0, 0))
                        nc.scalar.activation(
                            ex[0:msz, 0:2, si, n0:n0 + nsz],
                            ps[0:msz, 0:2, n0:n0 + nsz],
                            mybir.ActivationFunctionType.Exp)
                        if pj is not None:
                            pj(2 * si)
                            pj(2 * si + 1)
                    for hh in range(2):
                        h = 2 * p + hh
                        pav = avp.tile([128, 512], f32, tag="av", name="pav")
                        for si, (m0, msz) in enumerate(MSL):
                            n0, nsz = (85, NP - 85) if (mask_mode == "bc" and si > 0) else (0, NP)
                            nc.tensor.matmul(
                                pav[0:65, n0:n0 + nsz],
                                lhsT=vt[si][0:msz, h, :],
                                rhs=ex[0:msz, hh, si, n0:n0 + nsz],
                                start=(si == 0), stop=(si == 2))
                        if pj is not None:
                            pj(6 + hh)
                        nc.vector.tensor_copy(asb[:, h, :], pav[0:65, 0:NP])
                    # last batch: fold most of its own normalization into the
                    # attention phase (shrinks the exposed tail): heads 0-7
                    # staged at p3, quarters 8-11 at p5; pairs 0-5 normalized
                    # by the end of p7.
                    if b == BPC - 1:
                        if p == 3:
                            nc.sync.dma_start(staged_a[:],
                                              asb[64:65, 0:8, 0:NP])
                        elif p == 5:
                            emit_recip(self_item, part="a")
                            nc.sync.dma_start(staged_m[:],
                                              asb[64:65, 8:12, 0:NP])
                        elif p == 6:
                            emit_norm_pair(self_item, 0)
                            emit_norm_pair(self_item, 1)
                            emit_recip(self_item, part="m")
                        elif p == 7:
                            emit_norm_pair(self_item, 2)
                            emit_norm_pair(self_item, 3)
                            emit_norm_pair(self_item, 4)
                            emit_norm_pair(self_item, 5)
                if b == BPC - 1:
                    nc.sync.dma_start(staged_t[:], asb[64:65, 12:16, 0:NP])
                else:
                    nc.sync.dma_start(staged[:], asb[64:65, 0:16, 0:NP])
                pending.append(self_item)

            # ---- tail: the last batch's remaining normalization + proj ----
            while pending:
                item = pending.pop(0)
                if "rec_a" in item:
                    # pairs 0-5 were normalized during the attention phase;
                    # fo0/fo1's first six c-steps run while the last quarter's
                    # reciprocal + sel matmuls complete
                    emit_recip(item, part="t")
                    pj0 = make_proj_steps(item, 0, tail=True)
                    pj1 = make_proj_steps(item, 1, tail=True)
                    for c in range(6):
                        pj0(c)
                    emit_norm_pair(item, 6)
                    for c in range(6):
                        pj1(c)
                    emit_norm_pair(item, 7)
                    for c in range(6, 8):
                        pj0(c)
                    for c in range(6, 8):
                        pj1(c)
                    for p in range(2, 8):
                        pj = make_proj_steps(item, p, tail=True)
                        for c in range(8):
                            pj(c)
                else:
                    emit_recip(item)
                    for p in range(8):
                        emit_norm_pair(item, p)
                    for p in range(8):
                        pj = make_proj_steps(item, p, tail=True)
                        for c in range(8):
                            pj(c)
    nc.finalize()
    return nc


def _get_nc(mask_mode, use_qkv_bias):
    key = (mask_mode, use_qkv_bias)
    if key not in _cache:
        _cache[key] = _build(mask_mode, use_qkv_bias)
    return _cache[key]


def _bc_mask():
    seg = np.concatenate([np.full(s * s, i, dtype=np.int64) for i, s in enumerate(SCALES)])
    allow = seg[:, None] >= seg[None, :]
    return np.where(allow, 0.0, -1e9).astype(np.float32)[None, None]


def _prep_core_inputs(x, mask, qkv_w, qkv_b, proj_w, proj_b):
    tabs, ones = _host_tables()
    mf = mask.astype(np.float32)
    if not np.any(mf != 0):
        mask_mode = "none"
    elif np.array_equal(mf, _bc_mask()):
        mask_mode = "bc"
    else:
        mask_mode = "general"
    use_mask = mask_mode != "none"
    use_qb = bool(np.any(qkv_b != 0))

    wqkT = qkv_w.astype(np.float32).T.astype(BF16)      # [1024, 3072]
    wqk = np.ascontiguousarray(wqkT[:, :2048])
    wv = np.ascontiguousarray(wqkT[:, 2048:])
    wpT = np.ascontiguousarray(proj_w.astype(np.float32).T.astype(BF16))

    sel = np.zeros((16, 1024), np.float32)
    for h in range(16):
        sel[h, h * 64:(h + 1) * 64] = 1.0
    sel8 = np.zeros((8, 1024), np.float32)
    sel4 = np.zeros((4, 1024), np.float32)
    for cg in range(1024):
        p, r = cg // 128, cg % 128
        sel8[2 * (p % 4) + r // 64, cg] = 1.0
        sel4[2 * (p % 2) + r // 64, cg] = 1.0
    common = {"wqk": wqk, "wv": wv, "wp": wpT, "tabs": np.ascontiguousarray(tabs),
              "ones": np.ascontiguousarray(ones), "sel": sel.astype(BF16),
              "sel8": sel8.astype(BF16), "sel4": sel4.astype(BF16)}
    seg = np.concatenate([np.full(s * s, i, dtype=np.int64) for i, s in enumerate(SCALES)])
    if mask_mode == "bc":
        # staircase rank-4 additive mask (keys m<128 x queries n<85 only;
        # elsewhere block-causality holds by slice construction)
        u4 = np.zeros((4, 128), np.float32)
        w4 = np.zeros((4, 2, 85), np.float32)
        for j in range(4):
            u4[j, :] = np.where(seg[:128] >= j + 1, -1e9, 0.0)
            w4[j, :, :] = np.where(seg[:85] == j, 1.0, 0.0)[None, :]
        common["u4"] = u4.astype(BF16)
        common["w4"] = w4.reshape(4, 2 * 85).astype(BF16)
    elif mask_mode == "general":
        mT = mask[0, 0].astype(np.float32).T            # [keys, queries]
        mg = np.zeros((128, 2, 3, NP), np.float32)
        for s in range(3):
            ks = min(128, NTOK - s * 128)
            mg[:ks, :, s, :NTOK] = mT[s * 128:s * 128 + ks, None, :]
        common["mident"] = np.eye(128, dtype=np.float32).astype(BF16)
        common["maskg"] = mg.reshape(128, 2 * 3 * NP).astype(BF16)
    if use_qb:
        cos, sin = _rope_tables()
        sin2 = sin.copy(); sin2[:, 0::2] = -sin[:, 0::2]
        scale = 1.0 / np.sqrt(HD)
        qb_full = np.zeros((128, 16 * NP), np.float32)
        bq = qkv_b[:2048].astype(np.float32)
        for f in range(16):
            is_q = f < 8
            sc = scale if is_q else 1.0
            for hh in range(2):
                hvec = bq[f * 128 + hh * 64: f * 128 + (hh + 1) * 64]  # [64]
                hswap = hvec.reshape(-1, 2)[:, ::-1].reshape(-1)
                rb = cos * hvec[None, :] + sin2 * hswap[None, :]       # [341,64]
                qb_full[hh * 64:(hh + 1) * 64, f * NP: f * NP + NTOK] = sc * rb.T
        common["qb"] = qb_full.astype(BF16)
        common["vb"] = qkv_b[2048:].astype(np.float32).astype(BF16)[None, :]

    in_maps = []
    xf = x.astype(np.float32)
    for core in range(NCORES):
        xc = xf[core * BPC:(core + 1) * BPC]            # [4, 341, 1024]
        xp = np.zeros((BPC, NP, DIM), np.float32)
        xp[:, :NTOK, :] = xc
        xT = xp.reshape(BPC * NP, DIM).T                # [1024, 1376]
        m = dict(common)
        m["xt"] = np.ascontiguousarray(xT.astype(BF16))
        in_maps.append(m)
    return in_maps, mask_mode, use_qb


def kernel(x, mask, qkv_w, qkv_b, proj_w, proj_b, _trace=False):
    from concourse.bass_utils import run_bass_kernel_spmd
    x, mask, qkv_w, qkv_b, proj_w, proj_b = (
        np.asarray(t) for t in (x, mask, qkv_w, qkv_b, proj_w, proj_b))
    in_maps, mask_mode, use_qb = _prep_core_inputs(
        x, mask, qkv_w, qkv_b, proj_w, proj_b)
    nc = _get_nc(mask_mode, use_qb)
    res = run_bass_kernel_spmd(nc, in_maps, core_ids=list(range(NCORES)),
                               trace=_trace)
    out = np.empty((B, NTOK, DIM), np.float32)
    for core in range(NCORES):
        y = res.results[core]["out"].astype(np.float32)  # [DIM, BPC*NTOK]
        out[core * BPC:(core + 1) * BPC] = (
            y.T.reshape(BPC, NTOK, DIM))
    pb = proj_b.astype(np.float32)
    if np.any(pb != 0):
        out += pb[None, None, :]
    kernel._last_exec_time_ns = res.exec_time_ns
    if res.instructions_and_trace is not None:
        kernel._last_trace_path = res.instructions_and_trace[1]
    return out

